# revision 4
# baseline (speedup 1.0000x reference)
"""Trainium2 Bass kernel for ConvMultiHeadAttention (N=16, L=1024, E=512, H=8).

Data-parallel over batch: 8 NeuronCores x 2 batches each.

v2 design:
- Host-side layout prep: q/k/v passed pre-transposed [NB, E, L] fp16 and
  weights pre-transposed [P, EPO, E] fp16 (1/sqrt(D) folded into Wq), so the
  device does no transposes and no input casts.
- S^T logits per head-pair computed as two concurrent K=64 matmuls packed
  into row-groups (tile_position (0,0)/(64,0)) writing halves of one
  [128,1024] fp32 PSUM tile; one N=1024 Exp per tile on ScalarE.
- AV with an appended ones column (M=65) so the softmax denominator falls
  out as PSUM row 64; reciprocal on VectorE, partition-broadcast via a K=1
  outer-product matmul, normalization fused into the AV-PSUM evacuation.
- Output projection accumulates in PSUM, bias added during evacuation.
"""

import numpy as np
import concourse.bass as bass
import concourse.mybir as mybir
import concourse.tile as tile
from contextlib import ExitStack
from concourse import bacc

P = 128
L = 1024
E = 512
H = 8
D = 64
NB = 2            # batches per core
TT = L // P       # 8 token tiles per batch
EPO = E // P      # 4 e-subtiles
CH = L // E       # 2 query chunks of 512
HP = H // 2       # 4 head pairs
F32 = mybir.dt.float32
F16 = mybir.dt.float16
AF = mybir.ActivationFunctionType
ALU = mybir.AluOpType


def build(debug=False):
    nc = bacc.Bacc("TRN2", target_bir_lowering=False, debug=debug)
    qT_d = nc.dram_tensor("qT", [NB, E, L], F16, kind="ExternalInput").ap()
    kT_d = nc.dram_tensor("kT", [NB, E, L], F16, kind="ExternalInput").ap()
    vT_d = nc.dram_tensor("vT", [NB, E, L], F16, kind="ExternalInput").ap()
    wq_d = nc.dram_tensor("wqT", [P, EPO, E], F16, kind="ExternalInput").ap()
    wk_d = nc.dram_tensor("wkT", [P, EPO, E], F16, kind="ExternalInput").ap()
    wv_d = nc.dram_tensor("wvT", [P, EPO, E], F16, kind="ExternalInput").ap()
    wo_d = nc.dram_tensor("woT", [P, EPO, E], F16, kind="ExternalInput").ap()
    ones_d = nc.dram_tensor("ones1", [1, D], F16, kind="ExternalInput").ap()
    bo_d = nc.dram_tensor("bo_bcast", [P, E], F32, kind="ExternalInput").ap()
    out_d = nc.dram_tensor("out", [NB, L, E], F32, kind="ExternalOutput").ap()

    with tile.TileContext(nc) as tc, ExitStack() as ctx:
        consts = ctx.enter_context(tc.tile_pool(name="consts", bufs=1))
        xt_pool = ctx.enter_context(tc.tile_pool(name="xt", bufs=3))
        ht_pool = ctx.enter_context(tc.tile_pool(name="ht", bufs=2))
        vh_pool = ctx.enter_context(tc.tile_pool(name="vh", bufs=2))
        pts_pool = ctx.enter_context(tc.tile_pool(name="pts", bufs=32))
        st_pool = ctx.enter_context(tc.tile_pool(name="st", bufs=2))
        rc_pool = ctx.enter_context(tc.tile_pool(name="rc", bufs=2))
        rb_pool = ctx.enter_context(tc.tile_pool(name="rb", bufs=4))
        ot_pool = ctx.enter_context(tc.tile_pool(name="ot", bufs=3))
        ps_s = ctx.enter_context(tc.tile_pool(name="pss", bufs=2, space="PSUM"))
        ps_av = ctx.enter_context(tc.tile_pool(name="psav", bufs=2, space="PSUM"))
        ps_x = ctx.enter_context(tc.tile_pool(name="psx", bufs=2, space="PSUM"))

        # ---- constants ----
        wts = {}
        for wname, w_d in [("q", wq_d), ("k", wk_d), ("v", wv_d), ("o", wo_d)]:
            wt = consts.tile([P, EPO, E], F16, tag=f"wt_{wname}", name=f"wt_{wname}")
            nc.sync.dma_start(wt[:], w_d)
            wts[wname] = wt
        ones1 = consts.tile([1, D], F16, tag="ones1")
        nc.sync.dma_start(ones1[:], ones_d)
        bo_t = consts.tile([P, E], F32, tag="bo")
        nc.sync.dma_start(bo_t[:], bo_d)

        for b in range(NB):
            # ---- input loads (pre-transposed fp16) ----
            xts = {}
            for tname, x_d in [("q", qT_d), ("k", kT_d), ("v", vT_d)]:
                xt = xt_pool.tile([P, EPO, L], F16, tag="xt", name=f"xt_{tname}{b}")
                nc.sync.dma_start(
                    xt[:], x_d[b].rearrange("(epo p) t -> p epo t", p=P)
                )
                xts[tname] = xt

            # ---- Q/K projections -> transposed head layout [f, t] ----
            hts = {}
            for tname in ["q", "k"]:
                wt = wts[tname]
                xt = xts[tname]
                ht = ht_pool.tile([P, EPO, L], F16, tag=f"{tname}ht",
                                  name=f"{tname}ht{b}")
                for fpo in range(EPO):
                    for tch in range(CH):
                        ps = ps_x.tile([P, E], F32, tag="x", name="ps_qk")
                        for epo in range(EPO):
                            nc.tensor.matmul(
                                ps[:],
                                wt[:, epo, fpo * P:(fpo + 1) * P],
                                xt[:, epo, tch * E:(tch + 1) * E],
                                start=(epo == 0),
                                stop=(epo == EPO - 1),
                            )
                        nc.vector.tensor_copy(
                            ht[:, fpo, tch * E:(tch + 1) * E], ps[:]
                        )
                hts[tname] = ht
            qht, kht = hts["q"], hts["k"]

            # ---- V projection -> natural [t, h, d] with ones column ----
            vh = vh_pool.tile([P, TT, H, D + 1], F16, tag="vh", name=f"vh{b}")
            nc.vector.memset(vh[:], 1.0)  # ones col at [:,:,:,D]
            wt = wts["v"]
            xt = xts["v"]
            for tt in range(TT):
                ps = ps_x.tile([P, E], F32, tag="x", name="ps_v")
                for epo in range(EPO):
                    nc.tensor.matmul(
                        ps[:],
                        xt[:, epo, tt * P:(tt + 1) * P],
                        wt[:, epo, :],
                        start=(epo == 0),
                        stop=(epo == EPO - 1),
                    )
                nc.vector.tensor_copy(
                    vh[:, tt, :, 0:D],
                    ps[:].rearrange("p (h d) -> p h d", h=H),
                )

            # ---- attention ----
            stage = st_pool.tile([P, EPO, L], F16, tag="st", name=f"stage{b}")

            for hp in range(HP):
                # S^T for the head pair: two K=64 row-packed matmuls per
                # (key-tile, query-chunk) into one 2-bank psum tile.
                pts = {}
                for lt in range(TT):
                    for chq in range(CH):
                        ps = ps_s.tile([P, L], F32, tag="s", name="ps_s")
                        nc.tensor.matmul(
                            ps[:, 0:E],
                            kht[0:D, hp, lt * P:(lt + 1) * P],
                            qht[0:D, hp, chq * E:(chq + 1) * E],
                            start=True, stop=True,
                            tile_position=(0, 0),
                        )
                        nc.tensor.matmul(
                            ps[:, E:L],
                            kht[D:P, hp, lt * P:(lt + 1) * P],
                            qht[D:P, hp, chq * E:(chq + 1) * E],
                            start=True, stop=True,
                            tile_position=(D, 0),
                        )
                        pt = pts_pool.tile([P, L], F16, tag="pts", name="pt")
                        nc.scalar.activation(pt[:], ps[:], AF.Exp)
                        pts[(lt, chq)] = pt

                for hi in range(2):
                    h = 2 * hp + hi
                    hoff = D * hi
                    avps = []
                    r32h = rc_pool.tile([1, L], F32, tag="r32", bufs=3,
                                        name="r32h")
                    r16h = rc_pool.tile([1, L], F16, tag="r16", bufs=3,
                                        name="r16h")
                    for chq in range(CH):
                        avp = ps_av.tile([D + 1, E], F32, tag="av", name="ps_av")
                        for lt in range(TT):
                            nc.tensor.matmul(
                                avp[:],
                                vh[:, lt, h, :],
                                pts[(lt, chq)][:, hi * E:(hi + 1) * E],
                                start=(lt == 0),
                                stop=(lt == TT - 1),
                            )
                        nc.vector.reciprocal(
                            r32h[0:1, chq * E:(chq + 1) * E],
                            avp[D:D + 1, :],
                        )
                        avps.append(avp)
                    nc.vector.tensor_copy(r16h[:], r32h[:])
                    for chq in range(CH):
                        bcp = ps_x.tile([P, E], F32, tag="x", name="ps_bc")
                        nc.tensor.matmul(
                            bcp[0:D, :],
                            ones1[:],
                            r16h[0:1, chq * E:(chq + 1) * E],
                            start=True, stop=True,
                        )
                        rb = rb_pool.tile([D, E], F16, tag="rb", name="rb")
                        nc.vector.tensor_copy(rb[0:D, :], bcp[0:D, :])
                        nc.vector.tensor_tensor(
                            stage[hoff:hoff + D, hp, chq * E:(chq + 1) * E],
                            avps[chq][0:D, :],
                            rb[0:D, :],
                            ALU.mult,
                        )

            # ---- output projection + bias ----
            wt = wts["o"]
            for tt in range(TT):
                ps = ps_x.tile([P, E], F32, tag="x", name="ps_o")
                for hp in range(HP):
                    nc.tensor.matmul(
                        ps[:],
                        stage[:, hp, tt * P:(tt + 1) * P],
                        wt[:, hp, :],
                        start=(hp == 0),
                        stop=(hp == HP - 1),
                    )
                ot = ot_pool.tile([P, E], F32, tag="ot", name="ot")
                nc.vector.tensor_tensor(ot[:], ps[:], bo_t[:], ALU.add)
                nc.gpsimd.dma_start(out_d[b, tt * P:(tt + 1) * P, :], ot[:])

    nc.compile()
    return nc


_COMPILED = None


def _get_compiled():
    global _COMPILED
    if _COMPILED is None:
        _COMPILED = build()
    return _COMPILED


def prepare_in_maps(q, k, v, Wq, Wk, Wv, Wo, bo, n_cores=8):
    """Host-side layout prep shared by kernel() and the test harness."""
    qT = np.ascontiguousarray(
        np.asarray(q, np.float32).transpose(0, 2, 1).astype(np.float16))
    kT = np.ascontiguousarray(
        np.asarray(k, np.float32).transpose(0, 2, 1).astype(np.float16))
    vT = np.ascontiguousarray(
        np.asarray(v, np.float32).transpose(0, 2, 1).astype(np.float16))

    def wprep(W, scale=1.0):
        # wt[p, epo, f] = W[f, epo*128 + p] * scale
        a = (np.asarray(W, np.float32).T * scale).astype(np.float16)  # [e, f]
        return np.ascontiguousarray(a.reshape(EPO, P, E).transpose(1, 0, 2))

    wqT = wprep(Wq, 1.0 / np.sqrt(D))
    wkT = wprep(Wk)
    wvT = wprep(Wv)
    woT = wprep(Wo)
    ones1 = np.ones((1, D), np.float16)
    bo_bcast = np.ascontiguousarray(
        np.broadcast_to(np.asarray(bo, np.float32), (P, E)))

    in_maps = []
    for c in range(n_cores):
        in_maps.append({
            "qT": np.ascontiguousarray(qT[c * NB:(c + 1) * NB]),
            "kT": np.ascontiguousarray(kT[c * NB:(c + 1) * NB]),
            "vT": np.ascontiguousarray(vT[c * NB:(c + 1) * NB]),
            "wqT": wqT, "wkT": wkT, "wvT": wvT, "woT": woT,
            "ones1": ones1, "bo_bcast": bo_bcast,
        })
    return in_maps


def kernel(q, k, v, Wq, Wk, Wv, Wo, bo):
    n_cores = 8
    nc = _get_compiled()
    in_maps = prepare_in_maps(q, k, v, Wq, Wk, Wv, Wo, bo, n_cores)
    from concourse.bass_utils import run_bass_kernel_spmd
    res = run_bass_kernel_spmd(nc, in_maps, core_ids=list(range(n_cores)))
    out = np.concatenate([res.results[c]["out"] for c in range(n_cores)], axis=0)
    return out.astype(np.float32)


# revision 10
# speedup vs baseline: 1.2529x; 1.2529x over previous
"""Trainium2 Bass kernel for ConvMultiHeadAttention (N=16, L=1024, E=512, H=8).

Data-parallel over batch: 8 NeuronCores x 2 batches each.

v2 design:
- Host-side layout prep: q/k/v passed pre-transposed [NB, E, L] fp16 and
  weights pre-transposed [P, EPO, E] fp16 (1/sqrt(D) folded into Wq), so the
  device does no transposes and no input casts.
- S^T logits per head-pair computed as two concurrent K=64 matmuls packed
  into row-groups (tile_position (0,0)/(64,0)) writing halves of one
  [128,1024] fp32 PSUM tile; one N=1024 Exp per tile on ScalarE.
- AV with an appended ones column (M=65) so the softmax denominator falls
  out as PSUM row 64; reciprocal on VectorE, partition-broadcast via a K=1
  outer-product matmul, normalization fused into the AV-PSUM evacuation.
- Output projection accumulates in PSUM, bias added during evacuation.
"""

import numpy as np
import concourse.bass as bass
import concourse.mybir as mybir
import concourse.tile as tile
from contextlib import ExitStack
from concourse import bacc

P = 128
L = 1024
E = 512
H = 8
D = 64
NB = 2            # batches per core
TT = L // P       # 8 token tiles per batch
EPO = E // P      # 4 e-subtiles
CH = L // E       # 2 query chunks of 512
HP = H // 2       # 4 head pairs
F32 = mybir.dt.float32
F16 = mybir.dt.float16
AF = mybir.ActivationFunctionType
ALU = mybir.AluOpType


def build(debug=False):
    nc = bacc.Bacc("TRN2", target_bir_lowering=False, debug=debug)
    qT_d = nc.dram_tensor("qT", [NB, E, L], F16, kind="ExternalInput").ap()
    kT_d = nc.dram_tensor("kT", [NB, E, L], F16, kind="ExternalInput").ap()
    vT_d = nc.dram_tensor("vT", [NB, E, L], F16, kind="ExternalInput").ap()
    wq_d = nc.dram_tensor("wqT", [P, EPO, E], F16, kind="ExternalInput").ap()
    wk_d = nc.dram_tensor("wkT", [P, EPO, E], F16, kind="ExternalInput").ap()
    wv_d = nc.dram_tensor("wvT", [P, EPO, E], F16, kind="ExternalInput").ap()
    wo_d = nc.dram_tensor("woT", [P, EPO, E], F16, kind="ExternalInput").ap()
    sel_d = nc.dram_tensor("sel2", [P, H * D], F16, kind="ExternalInput").ap()
    bo_d = nc.dram_tensor("bo_bcast", [P, E], F32, kind="ExternalInput").ap()
    out_d = nc.dram_tensor("out", [NB, L, E], F32, kind="ExternalOutput").ap()

    with tile.TileContext(nc) as tc, ExitStack() as ctx:
        consts = ctx.enter_context(tc.tile_pool(name="consts", bufs=1))
        xt_pool = ctx.enter_context(tc.tile_pool(name="xt", bufs=3))
        ht_pool = ctx.enter_context(tc.tile_pool(name="ht", bufs=2))
        vh_pool = ctx.enter_context(tc.tile_pool(name="vh", bufs=2))
        pts_pool = ctx.enter_context(tc.tile_pool(name="pts", bufs=32))
        st_pool = ctx.enter_context(tc.tile_pool(name="st", bufs=2))
        rc_pool = ctx.enter_context(tc.tile_pool(name="rc", bufs=2))
        ot_pool = ctx.enter_context(tc.tile_pool(name="ot", bufs=3))
        ps_s = ctx.enter_context(tc.tile_pool(name="pss", bufs=2, space="PSUM"))
        ps_av = ctx.enter_context(tc.tile_pool(name="psav", bufs=2, space="PSUM"))
        ps_x = ctx.enter_context(tc.tile_pool(name="psx", bufs=2, space="PSUM"))

        # ---- constants ----
        wts = {}
        for wname, w_d in [("q", wq_d), ("k", wk_d), ("v", wv_d), ("o", wo_d)]:
            wt = consts.tile([P, EPO, E], F16, tag=f"wt_{wname}", name=f"wt_{wname}")
            nc.sync.dma_start(wt[:], w_d)
            wts[wname] = wt
        sel2 = consts.tile([P, H * D], F16, tag="sel2")
        nc.sync.dma_start(sel2[:], sel_d)
        bo_t = consts.tile([P, E], F32, tag="bo")
        nc.sync.dma_start(bo_t[:], bo_d)

        for b in range(NB):
            # ---- input loads (pre-transposed fp16) ----
            xts = {}
            for tname, x_d in [("q", qT_d), ("k", kT_d), ("v", vT_d)]:
                xt = xt_pool.tile([P, EPO, L], F16, tag="xt", name=f"xt_{tname}{b}")
                nc.sync.dma_start(
                    xt[:], x_d[b].rearrange("(epo p) t -> p epo t", p=P)
                )
                xts[tname] = xt

            # ---- Q/K projections -> transposed head layout [f, t] ----
            hts = {}
            for tname in ["q", "k"]:
                wt = wts[tname]
                xt = xts[tname]
                ht = ht_pool.tile([P, EPO, L], F16, tag=f"{tname}ht",
                                  name=f"{tname}ht{b}")
                for fpo in range(EPO):
                    for tch in range(CH):
                        ps = ps_x.tile([P, E], F32, tag="x", name="ps_qk")
                        for epo in range(EPO):
                            nc.tensor.matmul(
                                ps[:],
                                wt[:, epo, fpo * P:(fpo + 1) * P],
                                xt[:, epo, tch * E:(tch + 1) * E],
                                start=(epo == 0),
                                stop=(epo == EPO - 1),
                            )
                        nc.vector.tensor_copy(
                            ht[:, fpo, tch * E:(tch + 1) * E], ps[:]
                        )
                hts[tname] = ht
            qht, kht = hts["q"], hts["k"]

            # ---- V projection -> natural [t, h, d] with ones column ----
            vh = vh_pool.tile([P, TT, H, D + 1], F16, tag="vh", name=f"vh{b}")
            nc.vector.memset(vh[:], 1.0)  # ones col at [:,:,:,D]
            wt = wts["v"]
            xt = xts["v"]
            for tt in range(TT):
                ps = ps_x.tile([P, E], F32, tag="x", name="ps_v")
                for epo in range(EPO):
                    nc.tensor.matmul(
                        ps[:],
                        xt[:, epo, tt * P:(tt + 1) * P],
                        wt[:, epo, :],
                        start=(epo == 0),
                        stop=(epo == EPO - 1),
                    )
                nc.vector.tensor_copy(
                    vh[:, tt, :, 0:D],
                    ps[:].rearrange("p (h d) -> p h d", h=H),
                )

            # ---- attention ----
            stage = st_pool.tile([P, EPO, L], F16, tag="st", name=f"stage{b}")
            denom = rc_pool.tile([P, 2, L], F32, tag="dn", name=f"denom{b}")
            nc.vector.memset(denom[:], 1.0)
            recip = rc_pool.tile([P, 2, L], F16, tag="rcp", name=f"recip{b}")

            for hp in range(HP):
                # S^T for the head pair: two K=64 row-packed matmuls per
                # (key-tile, query-chunk) into one 2-bank psum tile.
                pts = {}
                for lt in range(TT):
                    for chq in range(CH):
                        ps = ps_s.tile([P, L], F32, tag="s", name="ps_s")
                        nc.tensor.matmul(
                            ps[:, 0:E],
                            kht[0:D, hp, lt * P:(lt + 1) * P],
                            qht[0:D, hp, chq * E:(chq + 1) * E],
                            start=True, stop=True,
                            tile_position=(0, 0),
                        )
                        nc.tensor.matmul(
                            ps[:, E:L],
                            kht[D:P, hp, lt * P:(lt + 1) * P],
                            qht[D:P, hp, chq * E:(chq + 1) * E],
                            start=True, stop=True,
                            tile_position=(D, 0),
                        )
                        pt = pts_pool.tile([P, L], F16, tag="pts", name="pt")
                        nc.scalar.activation(pt[:], ps[:], AF.Exp)
                        pts[(lt, chq)] = pt

                for hi in range(2):
                    h = 2 * hp + hi
                    hoff = D * hi
                    for chq in range(CH):
                        avp = ps_av.tile([D + 1, E], F32, tag="av", name="ps_av")
                        for lt in range(TT):
                            nc.tensor.matmul(
                                avp[:],
                                vh[:, lt, h, :],
                                pts[(lt, chq)][:, hi * E:(hi + 1) * E],
                                start=(lt == 0),
                                stop=(lt == TT - 1),
                            )
                        # raw O^T slice out; denominator row to collect tile
                        nc.vector.tensor_copy(
                            stage[hoff:hoff + D, hp, chq * E:(chq + 1) * E],
                            avp[0:D, :],
                        )
                        nc.vector.tensor_copy(
                            denom[32 * (h % 4):32 * (h % 4) + 1, h // 4,
                                  chq * E:(chq + 1) * E],
                            avp[D:D + 1, :],
                        )

                if hp % 2 == 1:
                    # heads 4*half .. 4*half+3 collected -> recip of this half
                    half = hp // 2
                    nc.scalar.activation(
                        denom[:, half, :], denom[:, half, :], AF.Ln)
                    nc.scalar.activation(
                        recip[:, half, :], denom[:, half, :], AF.Exp,
                        scale=-1.0)
                    for h in range(4 * half, 4 * half + 4):
                        hoff = D * (h % 2)
                        hpp = h // 2
                        for chq in range(CH):
                            psb = ps_x.tile([P, E], F32, tag="x", name="ps_bc")
                            nc.tensor.matmul(
                                psb[0:D, :],
                                sel2[:, h * D:(h + 1) * D],
                                recip[:, half, chq * E:(chq + 1) * E],
                                start=True, stop=True,
                            )
                            nc.vector.tensor_tensor(
                                stage[hoff:hoff + D, hpp,
                                      chq * E:(chq + 1) * E],
                                psb[0:D, :],
                                stage[hoff:hoff + D, hpp,
                                      chq * E:(chq + 1) * E],
                                ALU.mult,
                            )

            # ---- output projection + bias ----
            wt = wts["o"]
            for tt in range(TT):
                ps = ps_x.tile([P, E], F32, tag="x", name="ps_o")
                for hp in range(HP):
                    nc.tensor.matmul(
                        ps[:],
                        stage[:, hp, tt * P:(tt + 1) * P],
                        wt[:, hp, :],
                        start=(hp == 0),
                        stop=(hp == HP - 1),
                    )
                ot = ot_pool.tile([P, E], F32, tag="ot", name="ot")
                nc.vector.tensor_tensor(ot[:], ps[:], bo_t[:], ALU.add)
                nc.gpsimd.dma_start(out_d[b, tt * P:(tt + 1) * P, :], ot[:])

    nc.compile()
    return nc


_COMPILED = None


def _get_compiled():
    global _COMPILED
    if _COMPILED is None:
        _COMPILED = build()
    return _COMPILED


def prepare_in_maps(q, k, v, Wq, Wk, Wv, Wo, bo, n_cores=8):
    """Host-side layout prep shared by kernel() and the test harness."""
    qT = np.ascontiguousarray(
        np.asarray(q, np.float32).transpose(0, 2, 1).astype(np.float16))
    kT = np.ascontiguousarray(
        np.asarray(k, np.float32).transpose(0, 2, 1).astype(np.float16))
    vT = np.ascontiguousarray(
        np.asarray(v, np.float32).transpose(0, 2, 1).astype(np.float16))

    def wprep(W, scale=1.0):
        # wt[p, epo, f] = W[f, epo*128 + p] * scale
        a = (np.asarray(W, np.float32).T * scale).astype(np.float16)  # [e, f]
        return np.ascontiguousarray(a.reshape(EPO, P, E).transpose(1, 0, 2))

    wqT = wprep(Wq, 1.0 / np.sqrt(D))
    wkT = wprep(Wk)
    wvT = wprep(Wv)
    woT = wprep(Wo)
    sel2 = np.zeros((P, H * D), np.float16)
    for h in range(H):
        sel2[32 * (h % 4), h * D:(h + 1) * D] = 1.0
    bo_bcast = np.ascontiguousarray(
        np.broadcast_to(np.asarray(bo, np.float32), (P, E)))

    in_maps = []
    for c in range(n_cores):
        in_maps.append({
            "qT": np.ascontiguousarray(qT[c * NB:(c + 1) * NB]),
            "kT": np.ascontiguousarray(kT[c * NB:(c + 1) * NB]),
            "vT": np.ascontiguousarray(vT[c * NB:(c + 1) * NB]),
            "wqT": wqT, "wkT": wkT, "wvT": wvT, "woT": woT,
            "sel2": sel2, "bo_bcast": bo_bcast,
        })
    return in_maps


def kernel(q, k, v, Wq, Wk, Wv, Wo, bo):
    n_cores = 8
    nc = _get_compiled()
    in_maps = prepare_in_maps(q, k, v, Wq, Wk, Wv, Wo, bo, n_cores)
    from concourse.bass_utils import run_bass_kernel_spmd
    res = run_bass_kernel_spmd(nc, in_maps, core_ids=list(range(n_cores)))
    out = np.concatenate([res.results[c]["out"] for c in range(n_cores)], axis=0)
    return out.astype(np.float32)


# revision 11
# speedup vs baseline: 1.3163x; 1.0506x over previous
"""Trainium2 Bass kernel for ConvMultiHeadAttention (N=16, L=1024, E=512, H=8).

Data-parallel over batch: 8 NeuronCores x 2 batches each.

Design:
- Host-side layout prep: q/k/v passed pre-transposed [NB, E, L] fp16 and
  weights pre-transposed [P, EPO, E] fp16 (1/sqrt(D) folded into Wq), so the
  device does no transposes and no input casts.
- S^T logits per head-pair computed as two concurrent K=64 matmuls packed
  into row-groups (tile_position (0,0)/(64,0)) writing halves of one
  [128,1024] fp32 PSUM tile; one N=1024 Exp per tile on ScalarE.
- AV with an appended ones column (M=65) so the softmax denominator falls
  out as PSUM row 64; denominators collected per half-batch, recip via
  Ln+Exp(-x) on ScalarE (same table set as the softmax Exp), broadcast
  across partitions with a selector matmul, normalization as an in-place
  multiply on the staged O^T.
- Emission is software-pipelined across the two batches (batch 1 prep is
  emitted mid-batch-0 attention) and AV alternates its two chunk PSUM banks
  to avoid same-bank accumulate stalls.
"""

import numpy as np
import concourse.bass as bass
import concourse.mybir as mybir
import concourse.tile as tile
from contextlib import ExitStack
from concourse import bacc

P = 128
L = 1024
E = 512
H = 8
D = 64
NB = 2            # batches per core
TT = L // P       # 8 token tiles per batch
EPO = E // P      # 4 e-subtiles
CH = L // E       # 2 query chunks of 512
HP = H // 2       # 4 head pairs
F32 = mybir.dt.float32
F16 = mybir.dt.float16
AF = mybir.ActivationFunctionType
ALU = mybir.AluOpType


def build(debug=False):
    nc = bacc.Bacc("TRN2", target_bir_lowering=False, debug=debug)
    qT_d = nc.dram_tensor("qT", [NB, E, L], F16, kind="ExternalInput").ap()
    kT_d = nc.dram_tensor("kT", [NB, E, L], F16, kind="ExternalInput").ap()
    vT_d = nc.dram_tensor("vT", [NB, E, L], F16, kind="ExternalInput").ap()
    wq_d = nc.dram_tensor("wqT", [P, EPO, E], F16, kind="ExternalInput").ap()
    wk_d = nc.dram_tensor("wkT", [P, EPO, E], F16, kind="ExternalInput").ap()
    wv_d = nc.dram_tensor("wvT", [P, EPO, E], F16, kind="ExternalInput").ap()
    wo_d = nc.dram_tensor("woT", [P, EPO, E], F16, kind="ExternalInput").ap()
    sel_d = nc.dram_tensor("sel2", [P, H * D], F16, kind="ExternalInput").ap()
    bo_d = nc.dram_tensor("bo_bcast", [P, E], F32, kind="ExternalInput").ap()
    out_d = nc.dram_tensor("out", [NB, L, E], F32, kind="ExternalOutput").ap()

    with tile.TileContext(nc) as tc, ExitStack() as ctx:
        consts = ctx.enter_context(tc.tile_pool(name="consts", bufs=1))
        xt_pool = ctx.enter_context(tc.tile_pool(name="xt", bufs=3))
        ht_pool = ctx.enter_context(tc.tile_pool(name="ht", bufs=2))
        vh_pool = ctx.enter_context(tc.tile_pool(name="vh", bufs=2))
        pts_pool = ctx.enter_context(tc.tile_pool(name="pts", bufs=32))
        st_pool = ctx.enter_context(tc.tile_pool(name="st", bufs=2))
        rc_pool = ctx.enter_context(tc.tile_pool(name="rc", bufs=2))
        ot_pool = ctx.enter_context(tc.tile_pool(name="ot", bufs=3))
        ps_s = ctx.enter_context(tc.tile_pool(name="pss", bufs=2, space="PSUM"))
        ps_av = ctx.enter_context(tc.tile_pool(name="psav", bufs=2, space="PSUM"))
        ps_x = ctx.enter_context(tc.tile_pool(name="psx", bufs=2, space="PSUM"))

        wts = {}
        xts = [{} for _ in range(NB)]
        qkh = [{} for _ in range(NB)]
        vhs = [None] * NB
        stages = [None] * NB
        denoms = [None] * NB
        recips = [None] * NB
        ptss = [None] * NB

        def load_w(wname, w_d):
            wt = consts.tile([P, EPO, E], F16, tag=f"wt_{wname}",
                             name=f"wt_{wname}")
            nc.sync.dma_start(wt[:], w_d)
            wts[wname] = wt

        def load_x(b, tname, x_d):
            xt = xt_pool.tile([P, EPO, L], F16, tag="xt", name=f"xt_{tname}{b}")
            # chunked so queues fill evenly and first-needed data lands early
            src = x_d[b].rearrange("(epo p) t -> p epo t", p=P)
            for epo in range(EPO):
                nc.sync.dma_start(xt[:, epo, :], src[:, epo, :])
            xts[b][tname] = xt

        def emit_projQK(b, tname):
            wt = wts[tname]
            xt = xts[b][tname]
            ht = ht_pool.tile([P, EPO, L], F16, tag=f"{tname}ht",
                              name=f"{tname}ht{b}")
            for fpo in range(EPO):
                for tch in range(CH):
                    ps = ps_x.tile([P, E], F32, tag="x", name="ps_qk")
                    for epo in range(EPO):
                        nc.tensor.matmul(
                            ps[:],
                            wt[:, epo, fpo * P:(fpo + 1) * P],
                            xt[:, epo, tch * E:(tch + 1) * E],
                            start=(epo == 0),
                            stop=(epo == EPO - 1),
                        )
                    nc.vector.tensor_copy(
                        ht[:, fpo, tch * E:(tch + 1) * E], ps[:]
                    )
            qkh[b][tname] = ht

        def emit_projV(b):
            vh = vh_pool.tile([P, TT, H, D + 1], F16, tag="vh", name=f"vh{b}")
            nc.vector.memset(vh[:], 1.0)  # ones col at [:,:,:,D]
            wt = wts["v"]
            xt = xts[b]["v"]
            for tt in range(TT):
                ps = ps_x.tile([P, E], F32, tag="x", name="ps_v")
                for epo in range(EPO):
                    nc.tensor.matmul(
                        ps[:],
                        xt[:, epo, tt * P:(tt + 1) * P],
                        wt[:, epo, :],
                        start=(epo == 0),
                        stop=(epo == EPO - 1),
                    )
                nc.vector.tensor_copy(
                    vh[:, tt, :, 0:D],
                    ps[:].rearrange("p (h d) -> p h d", h=H),
                )
            vhs[b] = vh

        def attn_state(b):
            stages[b] = st_pool.tile([P, EPO, L], F16, tag="st",
                                     name=f"stage{b}")
            denoms[b] = rc_pool.tile([P, 2, L], F32, tag="dn", name=f"denom{b}")
            nc.vector.memset(denoms[b][:], 1.0)
            recips[b] = rc_pool.tile([P, 2, L], F16, tag="rcp", name=f"recip{b}")
            ptss[b] = {}

        def emit_pair(b, hp):
            qht, kht = qkh[b]["q"], qkh[b]["k"]
            vh = vhs[b]
            stage = stages[b]
            denom = denoms[b]
            pts = ptss[b]
            # S^T for the head pair: two concurrent K=64 row-packed matmuls
            # per (key-tile, query-chunk) into one 2-bank psum tile.
            for lt in range(TT):
                for chq in range(CH):
                    ps = ps_s.tile([P, L], F32, tag="s", name="ps_s")
                    nc.tensor.matmul(
                        ps[:, 0:E],
                        kht[0:D, hp, lt * P:(lt + 1) * P],
                        qht[0:D, hp, chq * E:(chq + 1) * E],
                        start=True, stop=True,
                        tile_position=(0, 0),
                    )
                    nc.tensor.matmul(
                        ps[:, E:L],
                        kht[D:P, hp, lt * P:(lt + 1) * P],
                        qht[D:P, hp, chq * E:(chq + 1) * E],
                        start=True, stop=True,
                        tile_position=(D, 0),
                    )
                    pt = pts_pool.tile([P, L], F16, tag="pts", name="pt")
                    nc.scalar.activation(pt[:], ps[:], AF.Exp)
                    pts[(hp, lt, chq)] = pt

            for hi in range(2):
                h = 2 * hp + hi
                hoff = D * hi
                avps = [ps_av.tile([D + 1, E], F32, tag="av", name="ps_av")
                        for _ in range(CH)]
                # alternate chunk banks so consecutive matmuls never
                # accumulate into the same psum bank back-to-back
                for lt in range(TT):
                    for chq in range(CH):
                        nc.tensor.matmul(
                            avps[chq][:],
                            vh[:, lt, h, :],
                            pts[(hp, lt, chq)][:, hi * E:(hi + 1) * E],
                            start=(lt == 0),
                            stop=(lt == TT - 1),
                        )
                for chq in range(CH):
                    avp = avps[chq]
                    nc.vector.tensor_copy(
                        stage[hoff:hoff + D, hp, chq * E:(chq + 1) * E],
                        avp[0:D, :],
                    )
                    nc.vector.tensor_copy(
                        denom[32 * (h % 4):32 * (h % 4) + 1, h // 4,
                              chq * E:(chq + 1) * E],
                        avp[D:D + 1, :],
                    )

        def emit_norm_half(b, half):
            # heads 4*half .. 4*half+3 are collected in denom[:, half, :]
            stage = stages[b]
            nc.scalar.activation(
                denoms[b][:, half, :], denoms[b][:, half, :], AF.Ln)
            nc.scalar.activation(
                recips[b][:, half, :], denoms[b][:, half, :], AF.Exp,
                scale=-1.0)
            sel2 = wts["sel2"]
            for h in range(4 * half, 4 * half + 4):
                hoff = D * (h % 2)
                hpp = h // 2
                for chq in range(CH):
                    psb = ps_x.tile([P, E], F32, tag="x", name="ps_bc")
                    nc.tensor.matmul(
                        psb[0:D, :],
                        sel2[:, h * D:(h + 1) * D],
                        recips[b][:, half, chq * E:(chq + 1) * E],
                        start=True, stop=True,
                    )
                    nc.vector.tensor_tensor(
                        stage[hoff:hoff + D, hpp, chq * E:(chq + 1) * E],
                        psb[0:D, :],
                        stage[hoff:hoff + D, hpp, chq * E:(chq + 1) * E],
                        ALU.mult,
                    )

        def emit_oproj(b):
            wt = wts["o"]
            stage = stages[b]
            for tt in range(TT):
                ps = ps_x.tile([P, E], F32, tag="x", name="ps_o")
                for hp in range(HP):
                    nc.tensor.matmul(
                        ps[:],
                        stage[:, hp, tt * P:(tt + 1) * P],
                        wt[:, hp, :],
                        start=(hp == 0),
                        stop=(hp == HP - 1),
                    )
                ot = ot_pool.tile([P, E], F32, tag="ot", name="ot")
                nc.vector.tensor_tensor(ot[:], ps[:], bo_t[:], ALU.add)
                nc.gpsimd.dma_start(out_d[b, tt * P:(tt + 1) * P, :], ot[:])

        # ---- emission: software-pipelined across the two batches ----
        load_x(0, "q", qT_d)
        load_x(0, "k", kT_d)
        load_w("q", wq_d)
        load_w("k", wk_d)
        load_x(0, "v", vT_d)
        load_w("v", wv_d)
        load_w("o", wo_d)
        sel2_t = consts.tile([P, H * D], F16, tag="sel2", name="sel2_t")
        nc.sync.dma_start(sel2_t[:], sel_d)
        wts["sel2"] = sel2_t
        bo_t = consts.tile([P, E], F32, tag="bo", name="bo_t")
        nc.sync.dma_start(bo_t[:], bo_d)

        emit_projQK(0, "q")
        emit_projQK(0, "k")
        emit_projV(0)
        attn_state(0)
        emit_pair(0, 0)
        emit_pair(0, 1)
        emit_norm_half(0, 0)
        load_x(1, "q", qT_d)
        load_x(1, "k", kT_d)
        load_x(1, "v", vT_d)
        emit_projQK(1, "q")
        emit_projQK(1, "k")
        emit_pair(0, 2)
        emit_projV(1)
        emit_pair(0, 3)
        emit_norm_half(0, 1)
        attn_state(1)
        emit_pair(1, 0)
        emit_oproj(0)
        emit_pair(1, 1)
        emit_norm_half(1, 0)
        emit_pair(1, 2)
        emit_pair(1, 3)
        emit_norm_half(1, 1)
        emit_oproj(1)

    nc.compile()
    return nc


_COMPILED = None


def _get_compiled():
    global _COMPILED
    if _COMPILED is None:
        _COMPILED = build()
    return _COMPILED


def prepare_in_maps(q, k, v, Wq, Wk, Wv, Wo, bo, n_cores=8):
    """Host-side layout prep shared by kernel() and the test harness."""
    qT = np.ascontiguousarray(
        np.asarray(q, np.float32).transpose(0, 2, 1).astype(np.float16))
    kT = np.ascontiguousarray(
        np.asarray(k, np.float32).transpose(0, 2, 1).astype(np.float16))
    vT = np.ascontiguousarray(
        np.asarray(v, np.float32).transpose(0, 2, 1).astype(np.float16))

    def wprep(W, scale=1.0):
        # wt[p, epo, f] = W[f, epo*128 + p] * scale
        a = (np.asarray(W, np.float32).T * scale).astype(np.float16)  # [e, f]
        return np.ascontiguousarray(a.reshape(EPO, P, E).transpose(1, 0, 2))

    wqT = wprep(Wq, 1.0 / np.sqrt(D))
    wkT = wprep(Wk)
    wvT = wprep(Wv)
    woT = wprep(Wo)
    sel2 = np.zeros((P, H * D), np.float16)
    for h in range(H):
        sel2[32 * (h % 4), h * D:(h + 1) * D] = 1.0
    bo_bcast = np.ascontiguousarray(
        np.broadcast_to(np.asarray(bo, np.float32), (P, E)))

    in_maps = []
    for c in range(n_cores):
        in_maps.append({
            "qT": np.ascontiguousarray(qT[c * NB:(c + 1) * NB]),
            "kT": np.ascontiguousarray(kT[c * NB:(c + 1) * NB]),
            "vT": np.ascontiguousarray(vT[c * NB:(c + 1) * NB]),
            "wqT": wqT, "wkT": wkT, "wvT": wvT, "woT": woT,
            "sel2": sel2, "bo_bcast": bo_bcast,
        })
    return in_maps


def kernel(q, k, v, Wq, Wk, Wv, Wo, bo):
    n_cores = 8
    nc = _get_compiled()
    in_maps = prepare_in_maps(q, k, v, Wq, Wk, Wv, Wo, bo, n_cores)
    from concourse.bass_utils import run_bass_kernel_spmd
    res = run_bass_kernel_spmd(nc, in_maps, core_ids=list(range(n_cores)))
    out = np.concatenate([res.results[c]["out"] for c in range(n_cores)], axis=0)
    return out.astype(np.float32)
